# revision 9
# baseline (speedup 1.0000x reference)
"""GCN encoder (2-layer, BN, residual) on 8 Trainium2 NeuronCores.

Sharding: nodes partitioned contiguously across 8 cores (6250 each). Edges
bucketed by dst shard on host (integer-only preprocessing: bucket/sort/pad
edge indices, degree counts via bincount). All float math runs on device:

  - per-node norm d_out=rsqrt(clip(outdeg,1)) folded into an fp16 copy of the
    gather table (x*d_out, AllGathered to every core)
  - SpMM: dma_gather of 128-edge chunks (rows->partitions) + one-hot selector
    matmul on PE accumulating m^T[feat, dst] in PSUM; selector built on DVE
    from iota==slot compare (exact 0/1 entries)
  - d_in applied via a broadcast matrix during PSUM evacuation
  - W matmul with W as the stationary operand keeps the [feat, dst] layout so
    BN (per-feature affine) uses per-partition ACT scale/bias + fused ReLU
  - BN stats: per-core partial sums + 1KB AllReduce
  - layer-2 table: h1*d_out cast fp16, AllGathered
"""

import sys

sys.path.insert(0, "/opt/trn_rl_repo")

import numpy as np

P = 128
N_CORES = 8
EPS = 1e-5

# compute dtype for gather tables / selectors / segment matmul
_F16 = "float16"

# dma_gather tuning (device crashes observed for very large single calls)
GATHER_SINGLE_PACKET = True
GATHER_MAX_IDX = 768  # max indices per dma_gather instruction
DMA_SCRATCH = 16384


def _cdiv(a, b):
    return -(-a // b)


# ---------------------------------------------------------------------------
# host-side integer preprocessing (indices only; no float arithmetic on data)
# ---------------------------------------------------------------------------


def _wrap_idx_image(idx_list):
    """int16 index list (len % 16 == 0) -> [128, len/16] SBUF image.

    dma_gather reads idx i from partition i%16, free slot i//16; the 16-row
    pattern must be replicated 8x across the 128 partitions (one per Q7 core).
    """
    n = idx_list.shape[0]
    assert n % 16 == 0
    img16 = idx_list.reshape(n // 16, 16).T  # [16, n/16]
    return np.tile(img16, (8, 1)).astype(np.int16)  # [128, n/16]


def _host_prep(src, dst, n_nodes):
    """Bucket edges by (dst shard, dst tile, src half); pad to uniform chunk
    capacities so all 8 cores run one identical program."""
    NC = N_CORES
    SH = n_nodes // NC
    assert SH * NC == n_nodes
    T = _cdiv(SH, P)
    SPLIT = n_nodes // 2
    assert SPLIT < 32768 and (n_nodes - SPLIT) <= 32768

    src = np.asarray(src, np.int64)
    dst = np.asarray(dst, np.int64)

    per_core = []
    CA = CB = 1
    for k in range(NC):
        m = (dst >= k * SH) & (dst < (k + 1) * SH)
        s = src[m]
        dl = dst[m] - k * SH
        t_idx = dl // P
        slot = dl % P
        half = (s >= SPLIT).astype(np.int64)
        idxval = np.where(half == 1, s - SPLIT, s)
        per_core.append((t_idx, half, idxval, slot))
        for t in range(T):
            tm = t_idx == t
            na = int(np.count_nonzero(tm & (half == 0)))
            nb = int(np.count_nonzero(tm & (half == 1)))
            CA = max(CA, _cdiv(na, P))
            CB = max(CB, _cdiv(nb, P))

    n_chunks = T * (CA + CB)
    pairs = [(2 * b, min(2 * b + 1, T - 1)) for b in range(_cdiv(T, 2))]

    cores = []
    for k in range(NC):
        t_idx, half, idxval, slot = per_core[k]
        A_idx = np.zeros((T, CA * P), np.int16)
        B_idx = np.zeros((T, CB * P), np.int16)
        slots = np.zeros((n_chunks, P), np.float32)
        valid = np.zeros((n_chunks, P), np.float32)
        for t in range(T):
            tm = t_idx == t
            for h, (Cc, buf) in enumerate(((CA, A_idx), (CB, B_idx))):
                hm = tm & (half == h)
                iv = idxval[hm]
                sl = slot[hm]
                n = iv.shape[0]
                buf[t, :n] = iv.astype(np.int16)
                base = t * (CA + CB) + (0 if h == 0 else CA)
                for c in range(Cc):
                    lo, hi = c * P, min((c + 1) * P, n)
                    if hi > lo:
                        slots[base + c, : hi - lo] = sl[lo:hi]
                        valid[base + c, : hi - lo] = 1.0

        # gather-call index images: one A call + one B call per tile pair
        imgs = []
        offs_a, offs_b = [], []
        col = 0
        for t0, t1 in pairs:
            tl = [t0] if t0 == t1 else [t0, t1]
            for h, (Cc, buf, offs) in enumerate(
                ((CA, A_idx, offs_a), (CB, B_idx, offs_b))
            ):
                lst = np.concatenate([buf[t] for t in tl])
                img = _wrap_idx_image(lst)
                offs.append((col, img.shape[1], len(tl) * Cc * P))
                col += img.shape[1]
                imgs.append(img)
        idx_img = np.concatenate(imgs, axis=1)  # [128, col]

        # degree counts (integers), tile-column layout [P, T], pad rows deg=1
        outdeg = np.bincount(src, minlength=n_nodes).astype(np.int64)
        indeg = np.bincount(dst, minlength=n_nodes).astype(np.int64)
        mine = slice(k * SH, (k + 1) * SH)

        def _cols(d):
            v = np.ones(T * P, np.float32)
            v[:SH] = d[mine].astype(np.float32)
            return v.reshape(T, P).T.copy()  # [P, T]

        cores.append(
            dict(
                idx_img=idx_img,
                slotT=slots.T.copy(),  # [P, n_chunks] f32
                validT=valid.T.copy(),
                deg_out=_cols(outdeg),
                deg_in=_cols(indeg),
                offs_a=offs_a,
                offs_b=offs_b,
            )
        )

    meta = dict(
        SH=SH,
        T=T,
        SPLIT=SPLIT,
        CA=CA,
        CB=CB,
        n_chunks=n_chunks,
        pairs=pairs,
        idx_cols=cores[0]["idx_img"].shape[1],
        n_nodes=n_nodes,
        # call offsets are identical across cores by construction
        offs_a=cores[0]["offs_a"],
        offs_b=cores[0]["offs_b"],
    )
    for c in cores[1:]:
        assert c["offs_a"] == meta["offs_a"] and c["offs_b"] == meta["offs_b"]
        assert c["idx_img"].shape == cores[0]["idx_img"].shape
    return meta, cores


# ---------------------------------------------------------------------------
# device program (identical on all cores; all data-dependence through SBUF)
# ---------------------------------------------------------------------------


def _build_program(meta):
    import concourse.bacc as bacc
    import concourse.bass as bass
    import concourse.tile as tile
    from concourse import mybir
    from concourse.masks import make_identity

    f32 = mybir.dt.float32
    f16 = getattr(mybir.dt, _F16)
    Alu = mybir.AluOpType
    Act = mybir.ActivationFunctionType

    SH, T, SPLIT = meta["SH"], meta["T"], meta["SPLIT"]
    CA, CB = meta["CA"], meta["CB"]
    NCH = meta["n_chunks"]
    NN = meta["n_nodes"]
    pairs = meta["pairs"]
    rows_of = lambda t: min(P, SH - t * P)

    nc = bacc.Bacc(
        "TRN2",
        target_bir_lowering=False,
        debug=False,
        num_devices=N_CORES,
        dynamic_dma_scratch_size=DMA_SCRATCH,
    )

    # ---- I/O -------------------------------------------------------------
    x_shard = nc.dram_tensor("x_shard", [SH, P], f32, kind="ExternalInput")
    W1_t = nc.dram_tensor("W1", [P, P], f32, kind="ExternalInput")
    W2_t = nc.dram_tensor("W2", [P, P], f32, kind="ExternalInput")
    gm1 = nc.dram_tensor("gamma1", [P, 1], f32, kind="ExternalInput")
    bt1 = nc.dram_tensor("beta1", [P, 1], f32, kind="ExternalInput")
    gm2 = nc.dram_tensor("gamma2", [P, 1], f32, kind="ExternalInput")
    bt2 = nc.dram_tensor("beta2", [P, 1], f32, kind="ExternalInput")
    iota_t = nc.dram_tensor("iota", [P, P], f16, kind="ExternalInput")
    idx_t = nc.dram_tensor("idx_img", [P, meta["idx_cols"]], mybir.dt.int16,
                           kind="ExternalInput")
    slot_t = nc.dram_tensor("slotT", [P, NCH], f32, kind="ExternalInput")
    val_t = nc.dram_tensor("validT", [P, NCH], f32, kind="ExternalInput")
    dego_t = nc.dram_tensor("deg_out", [P, T], f32, kind="ExternalInput")
    degi_t = nc.dram_tensor("deg_in", [P, T], f32, kind="ExternalInput")
    out_t = nc.dram_tensor("out", [SH, P], f32, kind="ExternalOutput")

    with tile.TileContext(nc) as tc:
        with (
            tc.tile_pool(name="cst", bufs=1) as cst,
            tc.tile_pool(name="big", bufs=1) as big,
            tc.tile_pool(name="gat", bufs=2) as gat,
            tc.tile_pool(name="wrk", bufs=3) as wrk,
            tc.tile_pool(name="ps", bufs=2, space="PSUM") as ps,
            tc.tile_pool(name="dram", bufs=1, space="DRAM") as dram,
        ):
            # ---- constants / static data --------------------------------
            ident = cst.tile([P, P], f32)
            make_identity(nc, ident[:])
            W1s = cst.tile([P, P], f32)
            W2s = cst.tile([P, P], f32)
            iota = cst.tile([P, P], f16)
            nc.sync.dma_start(W1s[:], W1_t[:])
            nc.sync.dma_start(W2s[:], W2_t[:])
            nc.sync.dma_start(iota[:], iota_t[:])
            idx_sb = cst.tile([P, meta["idx_cols"]], mybir.dt.int16)
            nc.sync.dma_start(idx_sb[:], idx_t[:])
            slot_sb = cst.tile([P, NCH], f32)
            val_sb = cst.tile([P, NCH], f32)
            nc.sync.dma_start(slot_sb[:], slot_t[:])
            nc.sync.dma_start(val_sb[:], val_t[:])
            gm1s = cst.tile([P, 1], f32)
            bt1s = cst.tile([P, 1], f32)
            gm2s = cst.tile([P, 1], f32)
            bt2s = cst.tile([P, 1], f32)
            nc.sync.dma_start(gm1s[:], gm1[:])
            nc.sync.dma_start(bt1s[:], bt1[:])
            nc.sync.dma_start(gm2s[:], gm2[:])
            nc.sync.dma_start(bt2s[:], bt2[:])

            # ---- degree normalizers (float math on device) --------------
            d_out = cst.tile([P, T], f32)
            d_in = cst.tile([P, T], f32)
            for deg_dram, d_sb in ((dego_t, d_out), (degi_t, d_in)):
                raw = wrk.tile([P, T], f32, tag="degraw")
                nc.sync.dma_start(raw[:], deg_dram[:])
                nc.vector.tensor_scalar_max(raw[:], raw[:], 1.0)
                nc.scalar.sqrt(raw[:], raw[:])
                nc.vector.reciprocal(d_sb[:], raw[:])

            # d_in broadcast rows: din_bc[:, t*P+j] = d_in[j, t] for all rows
            din_bc = big.tile([P, T * P], f32)
            for t in range(T):
                bc_ps = ps.tile([P, P], f32, tag="tp")
                nc.tensor.transpose(
                    out=bc_ps[:],
                    in_=d_in[:, t : t + 1].to_broadcast([P, P]),
                    identity=ident[:],
                )
                nc.vector.tensor_copy(din_bc[:, t * P : (t + 1) * P], bc_ps[:])

            # ---- fp16 normalized gather table for layer 1 ---------------
            x16_shard = dram.tile([SH, P], f16)
            x16_full = dram.tile([NN, P], f16, addr_space="Shared")
            for t in range(T):
                r = rows_of(t)
                xt = wrk.tile([P, P], f32, tag="xload")
                nc.sync.dma_start(xt[:r, :], x_shard[t * P : t * P + r, :])
                st = wrk.tile([P, P], f16, tag="xstage")
                nc.vector.tensor_scalar(
                    st[:r, :], xt[:r, :], d_out[:r, t : t + 1], None, Alu.mult
                )
                nc.sync.dma_start(x16_shard[t * P : t * P + r, :], st[:r, :])
            nc.gpsimd.collective_compute(
                "AllGather",
                Alu.bypass,
                replica_groups=[list(range(N_CORES))],
                ins=[x16_shard.opt()],
                outs=[x16_full.opt()],
            )

            # persistent stores
            hpre = big.tile([P, T * P], f32)   # pre-BN activations [feat, dst]
            h1 = big.tile([P, T * P], f32)     # post-BN/relu layer-1 output
            h16_shard = dram.tile([SH, P], f16)
            h16_full = dram.tile([NN, P], f16, addr_space="Shared")

            def gconv_layer(table_full, W_sb, s1_cols, s2_cols):
                """SpMM + W matmul; fills hpre and the per-tile stat columns."""
                srcA = table_full[0:SPLIT, :]
                srcB = table_full[SPLIT:NN, :]
                for ip, (t0, t1) in enumerate(pairs):
                    tl = [t0] if t0 == t1 else [t0, t1]
                    bufs = {}
                    for h, (Cc, offs, sv) in enumerate(
                        ((CA, meta["offs_a"], srcA), (CB, meta["offs_b"], srcB))
                    ):
                        col, wcols, nidx = offs[ip]
                        g = gat.tile([P, 2 * Cc, P], f16, tag=f"g{h}")
                        nch = nidx // P
                        step = max(1, GATHER_MAX_IDX // P)
                        for c0 in range(0, nch, step):
                            c1 = min(c0 + step, nch)
                            nc.gpsimd.dma_gather(
                                g[:, c0:c1, :],
                                sv,
                                idx_sb[:, col + c0 * 8 : col + c1 * 8],
                                (c1 - c0) * P,
                                (c1 - c0) * P,
                                P,
                                single_packet=GATHER_SINGLE_PACKET,
                            )
                        bufs[h] = g
                    for ti, t in enumerate(tl):
                        mT = ps.tile([P, P], f32, tag="mT")
                        cid0 = t * (CA + CB)
                        for c in range(CA + CB):
                            h, cc = (0, c) if c < CA else (1, c - CA)
                            Cc = CA if h == 0 else CB
                            sel = wrk.tile([P, P], f16, tag="sel", bufs=4)
                            cid = cid0 + c
                            nc.vector.tensor_scalar(
                                sel[:],
                                iota[:],
                                slot_sb[:, cid : cid + 1],
                                val_sb[:, cid : cid + 1],
                                Alu.is_equal,
                                Alu.mult,
                            )
                            nc.tensor.matmul(
                                out=mT[:],
                                lhsT=bufs[h][:, ti * Cc + cc, :],
                                rhs=sel[:],
                                start=(c == 0),
                                stop=(c == CA + CB - 1),
                            )
                        # evacuate with d_in column scaling
                        mTs = wrk.tile([P, P], f32, tag="mTs")
                        nc.vector.tensor_tensor(
                            out=mTs[:],
                            in0=mT[:],
                            in1=din_bc[:, t * P : (t + 1) * P],
                            op=Alu.mult,
                        )
                        hp = ps.tile([P, P], f32, tag="hp")
                        nc.tensor.matmul(
                            out=hp[:], lhsT=W_sb[:], rhs=mTs[:], start=True, stop=True
                        )
                        # evacuate + per-feature partial sums for BN
                        nc.vector.tensor_scalar(
                            hpre[:, t * P : (t + 1) * P],
                            hp[:],
                            1.0,
                            None,
                            Alu.mult,
                            Alu.add,
                            accum_out=s1_cols[:, t : t + 1],
                        )
                        sq = wrk.tile([P, P], f16, tag="sq")
                        nc.scalar.activation(
                            sq[:],
                            hpre[:, t * P : (t + 1) * P],
                            Act.Square,
                            accum_out=s2_cols[:, t : t + 1],
                        )

            def bn_coeffs(s1_cols, s2_cols, gam, bet, tag):
                """AllReduce partial sums -> per-feature scale a, shift c."""
                stats_in = dram.tile([P, 2], f32, name=f"stats_in_{tag}")
                stats_out = dram.tile(
                    [P, 2], f32, addr_space="Shared", name=f"stats_out_{tag}"
                )
                pack = wrk.tile([P, 2], f32, tag="pack")
                nc.vector.tensor_reduce(
                    pack[:, 0:1], s1_cols[:], axis=mybir.AxisListType.X, op=Alu.add
                )
                nc.vector.tensor_reduce(
                    pack[:, 1:2], s2_cols[:], axis=mybir.AxisListType.X, op=Alu.add
                )
                nc.sync.dma_start(stats_in[:], pack[:])
                nc.gpsimd.collective_compute(
                    "AllReduce",
                    Alu.add,
                    replica_groups=[list(range(N_CORES))],
                    ins=[stats_in.opt()],
                    outs=[stats_out.opt()],
                )
                glob = wrk.tile([P, 2], f32, tag="glob")
                nc.sync.dma_start(glob[:], stats_out[:])
                mo = wrk.tile([P, 4], f32, tag="mo")
                # mo: 0=mu 1=E[h^2] 2=var+eps 3=scratch
                nc.vector.tensor_scalar(mo[:, 0:2], glob[:], 1.0 / NN, None, Alu.mult)
                nc.vector.tensor_tensor(
                    out=mo[:, 3:4], in0=mo[:, 0:1], in1=mo[:, 0:1], op=Alu.mult
                )
                nc.vector.tensor_tensor(
                    out=mo[:, 2:3], in0=mo[:, 1:2], in1=mo[:, 3:4], op=Alu.subtract
                )
                nc.vector.tensor_scalar_add(mo[:, 2:3], mo[:, 2:3], EPS)
                nc.scalar.sqrt(mo[:, 2:3], mo[:, 2:3])
                a_c = cst.tile([P, 2], f32, name=f"a_c_{gam.name}")
                nc.vector.reciprocal(a_c[:, 0:1], mo[:, 2:3])
                nc.vector.tensor_tensor(
                    out=a_c[:, 0:1], in0=a_c[:, 0:1], in1=gam[:], op=Alu.mult
                )
                nc.vector.tensor_tensor(
                    out=a_c[:, 1:2], in0=a_c[:, 0:1], in1=mo[:, 0:1], op=Alu.mult
                )
                nc.vector.tensor_tensor(
                    out=a_c[:, 1:2], in0=bet[:], in1=a_c[:, 1:2], op=Alu.subtract
                )
                return a_c

            # ================= layer 1 =================
            s1a = cst.tile([P, T], f32)
            s2a = cst.tile([P, T], f32)
            gconv_layer(x16_full, W1s, s1a, s2a)
            ac1 = bn_coeffs(s1a, s2a, gm1s, bt1s, "l1")

            # BN + relu -> h1; build fp16 normalized layer-2 table
            for t in range(T):
                r = rows_of(t)
                nc.scalar.activation(
                    h1[:, t * P : (t + 1) * P],
                    hpre[:, t * P : (t + 1) * P],
                    Act.Relu,
                    bias=ac1[:, 1:2],
                    scale=ac1[:, 0:1],
                )
                tp = ps.tile([P, P], f32, tag="tp")
                nc.tensor.transpose(
                    out=tp[:], in_=h1[:, t * P : (t + 1) * P], identity=ident[:]
                )
                st = wrk.tile([P, P], f16, tag="xstage")
                nc.vector.tensor_scalar(
                    st[:r, :], tp[:r, :], d_out[:r, t : t + 1], None, Alu.mult
                )
                nc.sync.dma_start(h16_shard[t * P : t * P + r, :], st[:r, :])
            nc.gpsimd.collective_compute(
                "AllGather",
                Alu.bypass,
                replica_groups=[list(range(N_CORES))],
                ins=[h16_shard.opt()],
                outs=[h16_full.opt()],
            )

            # ================= layer 2 =================
            s1b = cst.tile([P, T], f32)
            s2b = cst.tile([P, T], f32)
            gconv_layer(h16_full, W2s, s1b, s2b)
            ac2 = bn_coeffs(s1b, s2b, gm2s, bt2s, "l2")

            for t in range(T):
                r = rows_of(t)
                h2 = wrk.tile([P, P], f32, tag="h2")
                nc.scalar.activation(
                    h2[:],
                    hpre[:, t * P : (t + 1) * P],
                    Act.Identity,
                    bias=ac2[:, 1:2],
                    scale=ac2[:, 0:1],
                )
                nc.vector.tensor_tensor(
                    out=h2[:], in0=h2[:], in1=h1[:, t * P : (t + 1) * P], op=Alu.add
                )
                nc.scalar.activation(h2[:], h2[:], Act.Relu)
                tp = ps.tile([P, P], f32, tag="tp")
                nc.tensor.transpose(out=tp[:], in_=h2[:], identity=ident[:])
                ot = wrk.tile([P, P], f32, tag="ostage")
                nc.vector.tensor_copy(ot[:r, :], tp[:r, :])
                nc.sync.dma_start(out_t[t * P : t * P + r, :], ot[:r, :])

    nc.compile()
    return nc


# ---------------------------------------------------------------------------


_CACHE = {}


def _get_program(meta):
    key = (meta["SH"], meta["T"], meta["CA"], meta["CB"], meta["idx_cols"])
    if key not in _CACHE:
        _CACHE[key] = _build_program(meta)
    return _CACHE[key]


def kernel(**inputs):
    x = np.asarray(inputs["x"], np.float32)
    src = np.asarray(inputs["src"])
    dst = np.asarray(inputs["dst"])
    n_nodes = x.shape[0]

    meta, cores = _host_prep(src, dst, n_nodes)
    nc = _get_program(meta)

    SH = meta["SH"]
    iota = np.tile(np.arange(P, dtype=np.float16), (P, 1))
    in_maps = []
    for k in range(N_CORES):
        c = cores[k]
        in_maps.append(
            {
                "x_shard": np.ascontiguousarray(x[k * SH : (k + 1) * SH]),
                "W1": np.asarray(inputs["W1"], np.float32),
                "W2": np.asarray(inputs["W2"], np.float32),
                "gamma1": np.asarray(inputs["gamma1"], np.float32).reshape(P, 1),
                "beta1": np.asarray(inputs["beta1"], np.float32).reshape(P, 1),
                "gamma2": np.asarray(inputs["gamma2"], np.float32).reshape(P, 1),
                "beta2": np.asarray(inputs["beta2"], np.float32).reshape(P, 1),
                "iota": iota,
                "idx_img": c["idx_img"],
                "slotT": c["slotT"],
                "validT": c["validT"],
                "deg_out": c["deg_out"],
                "deg_in": c["deg_in"],
            }
        )

    from concourse.bass_utils import run_bass_kernel_spmd

    res = run_bass_kernel_spmd(nc, in_maps, core_ids=list(range(N_CORES)))
    out = np.concatenate([res.results[k]["out"] for k in range(N_CORES)], axis=0)
    return out.astype(np.float32)
